# revision 1
# baseline (speedup 1.0000x reference)
"""Trainium2 Bass kernel for nn_ModelSimplest_11596411699489.

Model: 4D conv (valid, 13^4 kernel, 1->3 ch, 18^4 -> 6^4) + bias + relu
       -> flatten (3888) -> dense (3888->2) + bias -> softmax.  B=512.

Mapping: the conv is lowered to matmuls over (z,w)-plane Toeplitz blocks.
For each output block (ox,oy) and each (kx,ky) kernel-plane offset, the
contribution of input plane (ox+kx, oy+ky) to all 108 outputs
(co,oz,ow) of the block is a [324 x 108] structured (Toeplitz) matrix
multiply, contracted over plane positions and accumulated in PSUM over
all 169 (kx,ky) offsets.  Plane rows are chunked 324 -> 128+128+68
partitions.  Adjacent oy blocks are paired into one N=512 matmul via a
strided y-pair access pattern (one PSUM bank per pair), so each core
holds 3 pair + 3 single accumulators (4.5 banks) and runs a single
fully-unrolled pass with x-row/weight-tile prefetch a full kx ahead.

Sharding (8 cores): output (ox,oy) 6x6 grid split into 4 quadrants of
3x3 blocks; batch split in half.  core = 4*h + q, q in [0,4) quadrant,
h in [0,2) batch half.  Each core computes feats for its 9 blocks /
256 samples, its partial dense logits, then an AllReduce over the 4
cores sharing a batch half + softmax (replicated).  Host concatenates
the two batch halves from cores 0 and 4.

Weights/planes are fed as fp16 (11-bit mantissa); accumulation is fp32
in PSUM.  Dense + softmax are fp32.
"""

import sys

if "/opt/trn_rl_repo" not in sys.path:
    sys.path.insert(0, "/opt/trn_rl_repo")

import os

import numpy as np

USE_BF16 = os.environ.get("KERNEL_BF16", "0") == "1"
if USE_BF16:
    import ml_dtypes
    NP16 = ml_dtypes.bfloat16
else:
    NP16 = np.float16

B, S, KS, SO, COUT = 512, 18, 13, 6, 3
PLANE = S * S            # 324
PCHUNKS = 3              # plane rows padded to 3*128
NB = B // 2              # batch per core (half)
M = COUT * SO * SO       # 108 outputs per block (co,oz,ow)
NBLK = 9                 # 3x3 blocks per quadrant
NT = KS * KS             # 169 (kx,ky) tap-plane offsets

_cache = {}


def _build_nc():
    import concourse.mybir as mybir
    import concourse.tile as tile
    from concourse import bacc

    f16 = mybir.dt.bfloat16 if USE_BF16 else mybir.dt.float16
    f32 = mybir.dt.float32

    nc = bacc.Bacc(num_devices=8)

    # layouts match the SBUF tile layouts so every DMA is contiguous
    # xp rows: 324 plane rows, chunked 128+128+68 onto partitions
    xp_d = nc.dram_tensor("xp", [15, PLANE, 15, NB], f16, kind="ExternalInput")
    wt_d = nc.dram_tensor("wt", [NT, 128, PCHUNKS, 128], f16, kind="ExternalInput")
    wd_d = nc.dram_tensor("wd", [NBLK, 128, 2], f32, kind="ExternalInput")
    cb_d = nc.dram_tensor("cb", [128, 1], f32, kind="ExternalInput")
    db_d = nc.dram_tensor("db", [128, 2], f32, kind="ExternalInput")
    out_d = nc.dram_tensor("out", [NB, 2], f32, kind="ExternalOutput")

    with tile.TileContext(nc) as tc:
        with (
            tc.tile_pool(name="xrows", bufs=1) as xpool,
            tc.tile_pool(name="wpool", bufs=1) as wpool,
            tc.tile_pool(name="feats", bufs=1) as fpool,
            tc.tile_pool(name="small", bufs=1) as spool,
            tc.tile_pool(name="psum", bufs=1, space="PSUM") as ppool,
            tc.tile_pool(name="dram", bufs=1, space="DRAM") as dpool,
        ):
            # constants
            cb_t = spool.tile([128, 1], f32, tag="cb")
            nc.sync.dma_start(out=cb_t[:], in_=cb_d[:])
            db_t = spool.tile([128, 2], f32, tag="db")
            nc.sync.dma_start(out=db_t[:], in_=db_d[:])
            wd_ts = []
            for bi in range(NBLK):
                t = spool.tile([128, 2], f32, tag=f"wd{bi}")
                nc.sync.dma_start(out=t[:], in_=wd_d[bi])
                wd_ts.append(t)

            # accumulators: per block-row i, a y-pair (j=0,1) -> [128, 2*NB]
            # (one psum bank) and a single (j=2) -> [128, NB].  4.5 banks.
            pair_acc = [
                ppool.tile([128, 2 * NB], f32, tag=f"pacc{i}", name=f"pacc{i}")
                for i in range(3)
            ]
            sing_acc = [
                ppool.tile([128, NB], f32, tag=f"sacc{i}", name=f"sacc{i}")
                for i in range(3)
            ]

            xrows = {}
            # chunk partition counts: 324 = 128 + 128 + 68
            CPART = (128, 128, 68)

            def load_xrow(X, fine=False, mid=None):
                # tile free layout (c, y, b); DMAs on three queues,
                # each contiguous in DRAM and in SBUF free space
                t = xpool.tile([128, PCHUNKS, 15, NB], f16, tag="xrow", bufs=5)
                if fine:
                    # split by y-range so early-ky matmuls unblock as soon as
                    # their slice lands (region-granular RAW deps)
                    for n, (y0, y1) in enumerate(((0, 5), (5, 10), (10, 15))):
                        nc.sync.dma_start(
                            out=t[:, 0, y0:y1, :], in_=xp_d[X, 0:128, y0:y1]
                        )
                        nc.scalar.dma_start(
                            out=t[:, 1, y0:y1, :], in_=xp_d[X, 128:256, y0:y1]
                        )
                        nc.gpsimd.dma_start(
                            out=t[:68, 2, y0:y1, :], in_=xp_d[X, 256:PLANE, y0:y1]
                        )
                        if n == 0 and mid is not None:
                            mid()  # urgent small loads right after first piece
                else:
                    nc.sync.dma_start(out=t[:, 0, :, :], in_=xp_d[X, 0:128])
                    nc.scalar.dma_start(out=t[:, 1, :, :], in_=xp_d[X, 128:256])
                    nc.gpsimd.dma_start(out=t[:68, 2, :, :], in_=xp_d[X, 256:PLANE])
                xrows[X] = t

            wengs = [nc.sync, nc.scalar, nc.gpsimd]
            wtiles = {}

            def load_wts(kx):
                lst = []
                for ky in range(KS):
                    w_t = wpool.tile(
                        [128, PCHUNKS, 128], f16, tag="w", bufs=32, name=f"w{kx}_{ky}"
                    )
                    wengs[ky % 3].dma_start(out=w_t[:], in_=wt_d[kx * KS + ky])
                    lst.append(w_t)
                wtiles[kx] = lst

            # preload: first x-row with the kx=0 weights interleaved right
            # after its first y-piece, then the other two x-rows
            load_xrow(0, fine=True, mid=lambda: load_wts(0))
            load_xrow(1, fine=True)
            load_xrow(2, fine=True)
            for kx in range(KS):
                # prefetch next kx's weights and x-row (a full kx of slack)
                if kx + 1 < KS:
                    load_wts(kx + 1)
                if kx + 3 <= 14:
                    load_xrow(kx + 3)
                wts = wtiles.pop(kx)
                for i in range(3):
                    xr = xrows[i + kx]
                    for ky in range(KS):
                        w_t = wts[ky]
                        for c in range(PCHUNKS):
                            kp = CPART[c]
                            first = kx == 0 and ky == 0 and c == 0
                            last = kx == KS - 1 and ky == KS - 1 and c == PCHUNKS - 1
                            nc.tensor.matmul(
                                pair_acc[i][:, :],
                                lhsT=w_t[:kp, c, :],
                                rhs=xr[:kp, c, ky : ky + 2, :],
                                start=first,
                                stop=last,
                            )
                            nc.tensor.matmul(
                                sing_acc[i][:, :],
                                lhsT=w_t[:kp, c, :],
                                rhs=xr[:kp, c, ky + 2, :],
                                start=first,
                                stop=last,
                            )
                del xrows[kx]

            # evac + relu + bias; feats[bi] is an AP [128, NB] per block
            feats = []
            for i in range(3):
                pf = fpool.tile([128, 2 * NB], f32, tag=f"pfeat{i}", name=f"pf{i}")
                nc.scalar.activation(
                    pf[:],
                    pair_acc[i][:],
                    mybir.ActivationFunctionType.Relu,
                    bias=cb_t[:],
                    scale=1.0,
                )
                sf = fpool.tile([128, NB], f32, tag=f"sfeat{i}", name=f"sf{i}")
                nc.scalar.activation(
                    sf[:],
                    sing_acc[i][:],
                    mybir.ActivationFunctionType.Relu,
                    bias=cb_t[:],
                    scale=1.0,
                )
                feats += [(pf, 0), (pf, NB), (sf, 0)]

            # dense partials: logits[b, cls] = sum_f feats[f, b] * wd[f, cls]
            cc_in = dpool.tile([2, 128, 2], f32, tag="ccin")
            cc_out = dpool.tile([2, 128, 2], f32, tag="ccout")
            for hh in range(2):
                dacc = ppool.tile([128, 2], f32, tag="dacc", bufs=2)
                for bi in range(NBLK):
                    ft, off = feats[bi]
                    nc.tensor.matmul(
                        dacc[:, :],
                        lhsT=ft[:, off + hh * 128 : off + (hh + 1) * 128],
                        rhs=wd_ts[bi][:],
                        start=(bi == 0),
                        stop=(bi == NBLK - 1),
                    )
                lg = spool.tile([128, 2], f32, tag=f"lg{hh}")
                nc.vector.tensor_copy(lg[:], dacc[:])
                nc.sync.dma_start(out=cc_in[hh], in_=lg[:])

            nc.gpsimd.collective_compute(
                "AllReduce",
                mybir.AluOpType.add,
                replica_groups=[[0, 1, 2, 3], [4, 5, 6, 7]],
                ins=[cc_in.opt()],
                outs=[cc_out.opt()],
            )

            for hh in range(2):
                lr = spool.tile([128, 2], f32, tag=f"lr{hh}")
                nc.sync.dma_start(out=lr[:], in_=cc_out[hh])
                lb = spool.tile([128, 2], f32, tag=f"lb{hh}")
                nc.vector.tensor_add(lb[:], lr[:], db_t[:])
                ex = spool.tile([128, 2], f32, tag=f"ex{hh}")
                nc.scalar.activation(
                    ex[:], lb[:], mybir.ActivationFunctionType.Exp
                )
                sm = spool.tile([128, 1], f32, tag=f"sm{hh}")
                nc.vector.reduce_sum(sm[:], ex[:], axis=mybir.AxisListType.X)
                rc = spool.tile([128, 1], f32, tag=f"rc{hh}")
                nc.vector.reciprocal(rc[:], sm[:])
                pr = spool.tile([128, 2], f32, tag=f"pr{hh}")
                nc.vector.tensor_scalar_mul(pr[:], ex[:], rc[:])
                nc.sync.dma_start(
                    out=out_d[hh * 128 : (hh + 1) * 128, :], in_=pr[:]
                )

    nc.finalize()
    return nc


def _build_wt(conv_w):
    """[3,1,13,13,13,13] f32 -> [169, 3, 128, 128] f16 Toeplitz plane tiles."""
    c_idx = np.arange(PCHUNKS)[:, None]
    r_idx = np.arange(128)[None, :]
    p = c_idx * 128 + r_idx                      # [3,128] plane row id
    z = p // S
    w_ = p % S
    pvalid = p < PLANE
    m = np.arange(M)
    co = m // (SO * SO)
    oz = (m % (SO * SO)) // SO
    ow = m % SO
    dz = z[..., None] - oz[None, None, :]        # [3,128,108]
    dw = w_[..., None] - ow[None, None, :]
    valid = pvalid[..., None] & (dz >= 0) & (dz < KS) & (dw >= 0) & (dw < KS)
    dzc = np.clip(dz, 0, KS - 1)
    dwc = np.clip(dw, 0, KS - 1)
    cw = conv_w[:, 0]                            # [3,13,13,13,13]
    wt = np.zeros((NT, 128, PCHUNKS, 128), NP16)
    cob = np.broadcast_to(co[None, None, :], dz.shape)
    for kx in range(KS):
        for ky in range(KS):
            vals = cw[cob, kx, ky, dzc, dwc]     # [3(c),128(p),108(m)]
            wt[kx * KS + ky, :, :, :M] = (
                np.where(valid, vals, 0.0).astype(NP16).transpose(1, 0, 2)
            )
    return wt


def _build_inputs(x, conv_w, conv_b, dense_w, dense_b):
    x6 = np.ascontiguousarray(x.reshape(B, S, S, PLANE))
    wt = _build_wt(conv_w)

    cb = np.zeros((128, 1), np.float32)
    cb[:M, 0] = conv_b[np.arange(M) // (SO * SO)]
    db = np.tile(dense_b[None, :].astype(np.float32), (128, 1))

    m = np.arange(M)
    co = m // (SO * SO)
    oz = (m % (SO * SO)) // SO
    ow = m % SO

    in_maps = []
    for core in range(8):
        q, h = core % 4, core // 4
        qx0, qy0 = 3 * (q // 2), 3 * (q % 2)
        slab = x6[h * NB : (h + 1) * NB, qx0 : qx0 + 15, qy0 : qy0 + 15, :]
        t = np.transpose(slab, (1, 2, 3, 0)).astype(NP16)  # [15,15,324,NB]
        # -> [X, p, y, b]: each chunk's DMA is contiguous
        xp = np.ascontiguousarray(np.transpose(t, (0, 2, 1, 3)))

        wd = np.zeros((NBLK, 128, 2), np.float32)
        for bi in range(NBLK):
            ox, oy = qx0 + bi // 3, qy0 + bi % 3
            f = co * (SO**4) + ox * (SO**3) + oy * (SO**2) + oz * SO + ow
            wd[bi, :M, :] = dense_w[:, f].T
        in_maps.append({"xp": xp, "wt": wt, "wd": wd, "cb": cb, "db": db})
    return in_maps


def _run(in_maps, trace=False):
    from concourse.bass_utils import run_bass_kernel_spmd

    if "nc" not in _cache:
        _cache["nc"] = _build_nc()
    return run_bass_kernel_spmd(_cache["nc"], in_maps, list(range(8)), trace=trace)


def kernel(x, conv_w, conv_b, dense_w, dense_b, _trace=False):
    x = np.asarray(x, np.float32)
    conv_w = np.asarray(conv_w, np.float32)
    conv_b = np.asarray(conv_b, np.float32)
    dense_w = np.asarray(dense_w, np.float32)
    dense_b = np.asarray(dense_b, np.float32)

    in_maps = _build_inputs(x, conv_w, conv_b, dense_w, dense_b)
    res = _run(in_maps, trace=_trace)
    out = np.concatenate([res.results[0]["out"], res.results[4]["out"]], axis=0)
    if _trace:
        return out, res
    return out



# revision 2
# speedup vs baseline: 2.4082x; 2.4082x over previous
"""Trainium2 Bass kernel for nn_ModelSimplest_11596411699489.

Model: 4D conv (valid, 13^4 kernel, 1->3 ch, 18^4 -> 6^4) + bias + relu
       -> flatten (3888) -> dense (3888->2) + bias -> softmax.  B=512.

Mapping: conv lowered to fp8 DoubleRow matmuls over (z,w)-plane Toeplitz
blocks.  For each output position (ox,oy) and kernel-plane offset
(kx,ky), the contribution of input plane (ox+kx, oy+ky) to the 108
outputs (co,oz,ow) is a [324 x 108] Toeplitz matrix.  Plane rows are
chunked 324 = 3*108 partitions; the (kx,c,ky) tiles are flattened to
507 k-tiles of 108 rows and consumed two-at-a-time by fp8e4
MatmulPerfMode.DoubleRow matmuls (2 k-tiles contracted per streamed
column -> 2x MAC rate vs fp16).  A "hex" access pattern feeds all 6 oy
blocks of an ox-row in one N=384 matmul: rhs [108, 2(ktile), 6(oy), 64(b)]
with the oy dim striding the y-planes (same weight applies to block oy
at plane y=ky+oy).  All 6 ox-rows accumulate in parallel (6 PSUM banks),
254 DoubleRow matmuls each.

Sharding (8 cores): pure data parallel, 64 samples per core.  Each core
computes all 36 (ox,oy) blocks for its batch slice, then the dense
layer + softmax locally -- no collective.  Host concatenates the 8
[64, 2] outputs.

Quantization: x*16 and conv_w*64 in fp8 e4m3fn (descaled by 2^-10 in
the relu+bias activation), fp32 accumulation in PSUM.  Dense + softmax
in fp16/fp32.  End-to-end rel err ~1.4e-2 (gate 2e-2).
"""

import sys

if "/opt/trn_rl_repo" not in sys.path:
    sys.path.insert(0, "/opt/trn_rl_repo")

import numpy as np
import ml_dtypes

E4 = ml_dtypes.float8_e4m3fn

B, S, KS, SO, COUT = 512, 18, 13, 6, 3
NBC = B // 8             # 64 samples per core
P3 = 108                 # partition rows per plane chunk; 3*108 = 324
M = COUT * SO * SO       # 108 outputs per (ox,oy) block
NT = KS * 3 * KS         # 507 k-tiles (kx, c, ky)
NPAIR = (NT + 1) // 2    # 254 DoubleRow matmuls per ox-row

XSCALE, WSCALE = 16.0, 64.0
DESCALE = 1.0 / (XSCALE * WSCALE)

# x SBUF tile free-dim strides (elements): [X(18), c(3), y(18), b(64)]
XSTR = 3 * 18 * 64       # 3456 per X
CSTR = 18 * 64           # 1152 per c
YSTR = 64

_cache = {}


def _tile_order():
    """k-tile t -> (kx, c, ky), flattened kx-major for DMA-friendly order."""
    return [(kx, c, ky) for kx in range(KS) for c in range(3) for ky in range(KS)]


def _build_nc():
    import concourse.mybir as mybir
    import concourse.tile as tile
    from concourse import bacc
    from concourse.ap import AP

    f8 = mybir.dt.float8e4
    f16 = mybir.dt.float16
    f32 = mybir.dt.float32

    nc = bacc.Bacc(num_devices=8)

    xp_d = nc.dram_tensor("xp", [18, 3, P3, 18, NBC], f8, kind="ExternalInput")
    wt_d = nc.dram_tensor("wt", [P3, 2 * NPAIR, 128], f8, kind="ExternalInput")
    cb_d = nc.dram_tensor("cb", [128, 1], f32, kind="ExternalInput")
    wd_d = nc.dram_tensor("wd", [128, 36, 2], f16, kind="ExternalInput")
    db_d = nc.dram_tensor("db", [NBC, 2], f32, kind="ExternalInput")
    out_d = nc.dram_tensor("out", [NBC, 2], f32, kind="ExternalOutput")

    tiles = _tile_order()

    with tile.TileContext(nc) as tc:
        with (
            tc.tile_pool(name="xp", bufs=1) as xpool,
            tc.tile_pool(name="wp", bufs=1) as wpool,
            tc.tile_pool(name="fp", bufs=1) as fpool,
            tc.tile_pool(name="sp", bufs=1) as spool,
            tc.tile_pool(name="pp", bufs=1, space="PSUM") as ppool,
        ):
            xt = xpool.tile([P3, 18, 3, 18, NBC], f8, tag="x")
            wt = wpool.tile([P3, 2 * NPAIR, 128], f8, tag="w")

            # --- DMA schedule, in compute-need order, 3 queues ---
            qs = [nc.sync, nc.scalar, nc.gpsimd]
            qi = [0]

            def dma(out, in_):
                qs[qi[0] % 3].dma_start(out=out, in_=in_)
                qi[0] += 1

            # urgent: first weight pairs + first x slices (c0, X0..5, y0:8)
            dma(wt[:, 0:13, :], wt_d[:, 0:13, :])          # kx0 c0 tiles
            for X in range(6):
                dma(xt[:, X, 0, 0:8, :], xp_d[X, 0, :, 0:8, :])
            for X in range(6):
                dma(xt[:, X, 0, 8:18, :], xp_d[X, 0, :, 8:18, :])
            dma(wt[:, 13:39, :], wt_d[:, 13:39, :])        # kx0 c1+c2
            for c in (1, 2):
                for X in range(6):
                    dma(xt[:, X, c, :, :], xp_d[X, c])
            # steady state: weights chunk for kx, then x row X=kx+5
            for kx in range(1, KS):
                t0, t1 = kx * 39, (kx + 1) * 39 + (1 if kx == KS - 1 else 0)
                dma(wt[:, t0:t1, :], wt_d[:, t0:t1, :])
                X = kx + 5
                for c in range(3):
                    dma(xt[:, X, c, :, :], xp_d[X, c])

            # small constants (scalar queue, after head burst)
            cb_t = spool.tile([128, 1], f32, tag="cb")
            nc.scalar.dma_start(out=cb_t[:], in_=cb_d[:])
            wd_t = spool.tile([128, 36, 2], f16, tag="wd")
            nc.scalar.dma_start(out=wd_t[:], in_=wd_d[:])
            db_t = spool.tile([NBC, 2], f32, tag="db")
            nc.scalar.dma_start(out=db_t[:], in_=db_d[:])

            accs = [
                ppool.tile([128, 512], f32, tag=f"acc{ox}", name=f"acc{ox}")
                for ox in range(6)
            ]

            xfull = xt[:]
            pstride = xfull.ap[0][0]
            xtensor = xfull.tensor

            def xoff(X, c, ky):
                return X * XSTR + c * CSTR + ky * YSTR

            # --- conv: 254 DoubleRow matmuls per ox-row, 6 rows ---
            for i in range(NPAIR):
                kx0, c0, ky0 = tiles[2 * i]
                if 2 * i + 1 < NT:
                    kx1, c1, ky1 = tiles[2 * i + 1]
                else:
                    kx1 = None  # zero-padded final tile
                lhsT = wt[:, 2 * i : 2 * i + 2, :]
                for ox in range(6):
                    o0 = xoff(ox + kx0, c0, ky0)
                    kstride = (
                        xoff(ox + kx1, c1, ky1) - o0 if kx1 is not None else 0
                    )
                    rhs = AP(
                        xtensor,
                        o0,
                        [[pstride, P3], [kstride, 2], [YSTR, 6], [1, NBC]],
                    )
                    nc.tensor.matmul(
                        accs[ox][:, 0 : 6 * NBC],
                        lhsT=lhsT,
                        rhs=rhs,
                        start=(i == 0),
                        stop=(i == NPAIR - 1),
                        perf_mode=mybir.MatmulPerfMode.DoubleRow,
                    )

            # --- evac + relu + bias -> fp16 feats; dense accumulation ---
            feats = fpool.tile([128, 6, 6, NBC], f16, tag="feats")
            dacc = ppool.tile([NBC, 2], f32, tag="dacc")
            for ox in range(6):
                nc.scalar.activation(
                    feats[:, ox, :, :],
                    accs[ox][:, 0 : 6 * NBC],
                    mybir.ActivationFunctionType.Relu,
                    bias=cb_t[:],
                    scale=DESCALE,
                )
                for oy in range(6):
                    nc.tensor.matmul(
                        dacc[:, :],
                        lhsT=feats[:, ox, oy, :],
                        rhs=wd_t[:, ox * 6 + oy, :],
                        start=(ox == 0 and oy == 0),
                        stop=(ox == 5 and oy == 5),
                    )

            # --- + bias, softmax over the 2 classes, write out ---
            lb = spool.tile([NBC, 2], f32, tag="lb")
            nc.vector.tensor_add(lb[:], dacc[:], db_t[:])
            ex = spool.tile([NBC, 2], f32, tag="ex")
            nc.scalar.activation(ex[:], lb[:], mybir.ActivationFunctionType.Exp)
            sm = spool.tile([NBC, 1], f32, tag="sm")
            nc.vector.reduce_sum(sm[:], ex[:], axis=mybir.AxisListType.X)
            rc = spool.tile([NBC, 1], f32, tag="rc")
            nc.vector.reciprocal(rc[:], sm[:])
            pr = spool.tile([NBC, 2], f32, tag="pr")
            nc.vector.tensor_scalar_mul(pr[:], ex[:], rc[:])
            nc.sync.dma_start(out=out_d[:], in_=pr[:])

    nc.finalize()
    return nc


def _build_wt(conv_w):
    """conv_w [3,1,13,13,13,13] -> [108, 508, 128] fp8 k-tile stack."""
    p = np.arange(P3)
    m = np.arange(M)
    co = m // (SO * SO)
    oz = (m % (SO * SO)) // SO
    ow = m % SO
    cw = (conv_w[:, 0] * WSCALE).astype(np.float32)  # [3,13,13,13,13]

    wt = np.zeros((P3, 2 * NPAIR, 128), np.float32)
    for t, (kx, c, ky) in enumerate(_tile_order()):
        pg = c * P3 + p                       # plane row id in [0,324)
        z = pg // S
        w_ = pg % S
        dz = z[:, None] - oz[None, :]         # [108,108]
        dw = w_[:, None] - ow[None, :]
        valid = (dz >= 0) & (dz < KS) & (dw >= 0) & (dw < KS)
        vals = cw[
            np.broadcast_to(co[None, :], dz.shape),
            kx, ky,
            np.clip(dz, 0, KS - 1),
            np.clip(dw, 0, KS - 1),
        ]
        wt[:, t, :M] = np.where(valid, vals, 0.0)
    return wt.astype(E4)


def _build_inputs(x, conv_w, conv_b, dense_w, dense_b):
    wt = _build_wt(conv_w)

    m = np.arange(M)
    co = m // (SO * SO)
    oz = (m % (SO * SO)) // SO
    ow = m % SO

    cb = np.zeros((128, 1), np.float32)
    cb[:M, 0] = conv_b[co]

    # dense weights regrouped per (ox,oy) block: feat = co*6^4 + ox*6^3 +
    # oy*6^2 + oz*6 + ow
    wd = np.zeros((128, 36, 2), np.float16)
    for blk in range(36):
        ox, oy = blk // 6, blk % 6
        f = co * SO**4 + ox * SO**3 + oy * SO**2 + oz * SO + ow
        wd[:M, blk, :] = dense_w[:, f].T.astype(np.float16)

    db = np.tile(dense_b[None, :].astype(np.float32), (NBC, 1))

    in_maps = []
    for core in range(8):
        xs = x[NBC * core : NBC * (core + 1), 0]      # [64, X, Y, z, w]
        t = xs.transpose(1, 3, 4, 2, 0)               # [X, z, w, Y, b]
        t = t.reshape(S, 3, P3, S, NBC)               # [X, c, p, y, b]
        xq = np.ascontiguousarray(t * XSCALE).astype(E4)
        in_maps.append({"xp": xq, "wt": wt, "cb": cb, "wd": wd, "db": db})
    return in_maps


def _run(in_maps, trace=False):
    from concourse.bass_utils import run_bass_kernel_spmd

    if "nc" not in _cache:
        _cache["nc"] = _build_nc()
    return run_bass_kernel_spmd(_cache["nc"], in_maps, list(range(8)), trace=trace)


def kernel(x, conv_w, conv_b, dense_w, dense_b, _trace=False):
    x = np.asarray(x, np.float32)
    conv_w = np.asarray(conv_w, np.float32)
    conv_b = np.asarray(conv_b, np.float32)
    dense_w = np.asarray(dense_w, np.float32)
    dense_b = np.asarray(dense_b, np.float32)

    in_maps = _build_inputs(x, conv_w, conv_b, dense_w, dense_b)
    res = _run(in_maps, trace=_trace)
    out = np.concatenate([res.results[i]["out"] for i in range(8)], axis=0)
    if _trace:
        return out, res
    return out


# revision 4
# speedup vs baseline: 2.4363x; 1.0117x over previous
"""Trainium2 Bass kernel for nn_ModelSimplest_11596411699489.

Model: 4D conv (valid, 13^4 kernel, 1->3 ch, 18^4 -> 6^4) + bias + relu
       -> flatten (3888) -> dense (3888->2) + bias -> softmax.  B=512.

Mapping: conv lowered to fp8 DoubleRow matmuls over (z,w)-plane Toeplitz
blocks.  For each output position (ox,oy) and kernel-plane offset
(kx,ky), the contribution of input plane (ox+kx, oy+ky) to the 108
outputs (co,oz,ow) is a [324 x 108] Toeplitz matrix.  Plane rows are
chunked 324 = 3*108 partitions; the (kx,c,ky) tiles are flattened to
507 k-tiles of 108 rows and consumed two-at-a-time by fp8e4
MatmulPerfMode.DoubleRow matmuls (2 k-tiles contracted per streamed
column -> 2x MAC rate vs fp16).  A "hex" access pattern feeds all 6 oy
blocks of an ox-row in one N=384 matmul: rhs [108, 2(ktile), 6(oy), 64(b)]
with the oy dim striding the y-planes (same weight applies to block oy
at plane y=ky+oy).  All 6 ox-rows accumulate in parallel (6 PSUM banks),
254 DoubleRow matmuls each.

Sharding (8 cores): pure data parallel, 64 samples per core.  Each core
computes all 36 (ox,oy) blocks for its batch slice, then the dense
layer + softmax locally -- no collective.  Host concatenates the 8
[64, 2] outputs.

Quantization: x*16 and conv_w*64 in fp8 e4m3fn (descaled by 2^-10 in
the relu+bias activation), fp32 accumulation in PSUM.  Dense + softmax
in fp16/fp32.  End-to-end rel err ~1.4e-2 (gate 2e-2).
"""

import sys

if "/opt/trn_rl_repo" not in sys.path:
    sys.path.insert(0, "/opt/trn_rl_repo")

import numpy as np
import ml_dtypes

E4 = ml_dtypes.float8_e4m3fn

B, S, KS, SO, COUT = 512, 18, 13, 6, 3
NBC = B // 8             # 64 samples per core
P3 = 108                 # partition rows per plane chunk; 3*108 = 324
M = COUT * SO * SO       # 108 outputs per (ox,oy) block
NT = KS * 3 * KS         # 507 k-tiles (kx, c, ky)
NPAIR = (NT + 1) // 2    # 254 DoubleRow matmuls per ox-row

XSCALE, WSCALE = 16.0, 64.0
DESCALE = 1.0 / (XSCALE * WSCALE)

# x SBUF tile free-dim strides (elements): [X(18), c(3), y(18), b(64)]
XSTR = 3 * 18 * 64       # 3456 per X
CSTR = 18 * 64           # 1152 per c
YSTR = 64

_cache = {}


def _tile_order():
    """k-tile t -> (kx, c, ky), flattened kx-major for DMA-friendly order."""
    return [(kx, c, ky) for kx in range(KS) for c in range(3) for ky in range(KS)]


def _build_nc():
    import concourse.mybir as mybir
    import concourse.tile as tile
    from concourse import bacc
    from concourse.ap import AP

    f8 = mybir.dt.float8e4
    f16 = mybir.dt.float16
    f32 = mybir.dt.float32

    nc = bacc.Bacc(num_devices=8)

    xp_d = nc.dram_tensor("xp", [18, 3, P3, 18, NBC], f8, kind="ExternalInput")
    wt_d = nc.dram_tensor("wt", [P3, 2 * NPAIR, 128], f8, kind="ExternalInput")
    cb_d = nc.dram_tensor("cb", [128, 1], f32, kind="ExternalInput")
    wd_d = nc.dram_tensor("wd", [128, 36, 2], f16, kind="ExternalInput")
    db_d = nc.dram_tensor("db", [NBC, 2], f32, kind="ExternalInput")
    out_d = nc.dram_tensor("out", [NBC, 2], f32, kind="ExternalOutput")

    tiles = _tile_order()

    with tile.TileContext(nc) as tc:
        with (
            tc.tile_pool(name="xp", bufs=1) as xpool,
            tc.tile_pool(name="wp", bufs=1) as wpool,
            tc.tile_pool(name="fp", bufs=1) as fpool,
            tc.tile_pool(name="sp", bufs=1) as spool,
            tc.tile_pool(name="pp", bufs=1, space="PSUM") as ppool,
        ):
            xt = xpool.tile([P3, 18, 3, 18, NBC], f8, tag="x")
            wt = wpool.tile([P3, 2 * NPAIR, 128], f8, tag="w")

            # --- DMA schedule, wavefront-need order, 3 queues ---
            qs = [nc.sync, nc.scalar, nc.gpsimd]
            qi = [0]

            def dma(out, in_):
                qs[qi[0] % 3].dma_start(out=out, in_=in_)
                qi[0] += 1

            # head: first weight pair + first x slices, then wave order.
            # wave d uses x row X=d and (for group ox=0) weight chunk kx=d.
            dma(wt[:, 0:2, :], wt_d[:, 0:2, :])
            dma(xt[:, 0, 0, 0:8, :], xp_d[0, 0, :, 0:8, :])
            dma(wt[:, 2:13, :], wt_d[:, 2:13, :])
            dma(xt[:, 0, 0, 8:18, :], xp_d[0, 0, :, 8:18, :])
            dma(xt[:, 0, 1, :, :], xp_d[0, 1])
            dma(xt[:, 0, 2, :, :], xp_d[0, 2])
            dma(wt[:, 13:26, :], wt_d[:, 13:26, :])
            dma(wt[:, 26:39, :], wt_d[:, 26:39, :])
            for d in range(1, 18):
                for c in range(3):
                    dma(xt[:, d, c, :, :], xp_d[d, c])
                if d < KS:
                    # weight chunk kx=d, split in 3 for queue parallelism
                    t0 = d * 39
                    t1 = (d + 1) * 39 + (1 if d == KS - 1 else 0)
                    dma(wt[:, t0 : t0 + 13, :], wt_d[:, t0 : t0 + 13, :])
                    dma(wt[:, t0 + 13 : t0 + 26, :], wt_d[:, t0 + 13 : t0 + 26, :])
                    dma(wt[:, t0 + 26 : t1, :], wt_d[:, t0 + 26 : t1, :])

            # small constants (needed only at evac/dense time)
            cb_t = spool.tile([128, 1], f32, tag="cb")
            nc.scalar.dma_start(out=cb_t[:], in_=cb_d[:])
            wd_t = spool.tile([128, 36, 2], f16, tag="wd")
            nc.scalar.dma_start(out=wd_t[:], in_=wd_d[:])
            db_t = spool.tile([NBC, 2], f32, tag="db")
            nc.scalar.dma_start(out=db_t[:], in_=db_d[:])

            accs = [
                ppool.tile([128, 512], f32, tag=f"acc{ox}", name=f"acc{ox}")
                for ox in range(6)
            ]
            feats = fpool.tile([128, 6, 6, NBC], f16, tag="feats")
            dacc = ppool.tile([NBC, 2], f32, tag="dacc")
            warm = ppool.tile([128, NBC], f32, tag="warm")

            xfull = xt[:]
            pstride = xfull.ap[0][0]
            xtensor = xfull.tensor

            def xoff(X, c, ky):
                return X * XSTR + c * CSTR + ky * YSTR

            # PE p-state warmup during the head DMA: small DoubleRow matmuls
            # on the first weight pair + first x slice (~64 cols each).
            for i in range(40):
                rhs = AP(xtensor, 0, [[pstride, P3], [0, 2], [1, NBC]])
                nc.tensor.matmul(
                    warm[:, :],
                    lhsT=wt[:, 0:2, :],
                    rhs=rhs,
                    start=True,
                    stop=True,
                    perf_mode=mybir.MatmulPerfMode.DoubleRow,
                )

            # --- conv, wavefront order: wave d = all (ox, pair) with
            # ox + kx(pair's later tile) == d.  Each acc[ox] spans waves
            # ox..ox+12; its evac + dense overlap later waves. ---
            waves = [[] for _ in range(18)]
            for ox in range(6):
                for p in range(NPAIR):
                    kxb = tiles[min(2 * p + 1, NT - 1)][0]
                    waves[ox + kxb].append((p, ox))
            for w in waves:
                w.sort()

            def conv_mm(p, ox):
                kx0, c0, ky0 = tiles[2 * p]
                if 2 * p + 1 < NT:
                    kx1, c1, ky1 = tiles[2 * p + 1]
                    o0 = xoff(ox + kx0, c0, ky0)
                    kstride = xoff(ox + kx1, c1, ky1) - o0
                else:
                    o0 = xoff(ox + kx0, c0, ky0)
                    kstride = 0
                rhs = AP(
                    xtensor, o0,
                    [[pstride, P3], [kstride, 2], [YSTR, 6], [1, NBC]],
                )
                nc.tensor.matmul(
                    accs[ox][:, 0 : 6 * NBC],
                    lhsT=wt[:, 2 * p : 2 * p + 2, :],
                    rhs=rhs,
                    start=(p == 0),
                    stop=(p == NPAIR - 1),
                    perf_mode=mybir.MatmulPerfMode.DoubleRow,
                )

            def evac(ox):
                nc.scalar.activation(
                    feats[:, ox, :, :],
                    accs[ox][:, 0 : 6 * NBC],
                    mybir.ActivationFunctionType.Relu,
                    bias=cb_t[:],
                    scale=DESCALE,
                )

            def dense(ox):
                for oy in range(6):
                    nc.tensor.matmul(
                        dacc[:, :],
                        lhsT=feats[:, ox, oy, :],
                        rhs=wd_t[:, ox * 6 + oy, :],
                        start=(ox == 0 and oy == 0),
                        stop=(ox == 5 and oy == 5),
                    )

            pending = []  # [ox, countdown]: inject dense ~8 matmuls post-evac
            for d in range(18):
                for p, ox in waves[d]:
                    conv_mm(p, ox)
                    for e in pending:
                        e[1] -= 1
                    if pending and pending[0][1] <= 0:
                        dense(pending.pop(0)[0])
                    if p == NPAIR - 1:
                        evac(ox)  # relu-evac runs under later conv matmuls
                        pending.append([ox, 8])
            for ox, _ in pending:
                dense(ox)

            # --- + bias, softmax over the 2 classes, write out ---
            lb = spool.tile([NBC, 2], f32, tag="lb")
            nc.vector.tensor_add(lb[:], dacc[:], db_t[:])
            ex = spool.tile([NBC, 2], f32, tag="ex")
            nc.scalar.activation(ex[:], lb[:], mybir.ActivationFunctionType.Exp)
            sm = spool.tile([NBC, 1], f32, tag="sm")
            nc.vector.reduce_sum(sm[:], ex[:], axis=mybir.AxisListType.X)
            rc = spool.tile([NBC, 1], f32, tag="rc")
            nc.vector.reciprocal(rc[:], sm[:])
            pr = spool.tile([NBC, 2], f32, tag="pr")
            nc.vector.tensor_scalar_mul(pr[:], ex[:], rc[:])
            nc.sync.dma_start(out=out_d[:], in_=pr[:])

    nc.finalize()
    return nc


def _build_wt(conv_w):
    """conv_w [3,1,13,13,13,13] -> [108, 508, 128] fp8 k-tile stack."""
    p = np.arange(P3)
    m = np.arange(M)
    co = m // (SO * SO)
    oz = (m % (SO * SO)) // SO
    ow = m % SO
    cw = (conv_w[:, 0] * WSCALE).astype(np.float32)  # [3,13,13,13,13]

    wt = np.zeros((P3, 2 * NPAIR, 128), np.float32)
    for t, (kx, c, ky) in enumerate(_tile_order()):
        pg = c * P3 + p                       # plane row id in [0,324)
        z = pg // S
        w_ = pg % S
        dz = z[:, None] - oz[None, :]         # [108,108]
        dw = w_[:, None] - ow[None, :]
        valid = (dz >= 0) & (dz < KS) & (dw >= 0) & (dw < KS)
        vals = cw[
            np.broadcast_to(co[None, :], dz.shape),
            kx, ky,
            np.clip(dz, 0, KS - 1),
            np.clip(dw, 0, KS - 1),
        ]
        wt[:, t, :M] = np.where(valid, vals, 0.0)
    return wt.astype(E4)


def _build_inputs(x, conv_w, conv_b, dense_w, dense_b):
    wt = _build_wt(conv_w)

    m = np.arange(M)
    co = m // (SO * SO)
    oz = (m % (SO * SO)) // SO
    ow = m % SO

    cb = np.zeros((128, 1), np.float32)
    cb[:M, 0] = conv_b[co]

    # dense weights regrouped per (ox,oy) block: feat = co*6^4 + ox*6^3 +
    # oy*6^2 + oz*6 + ow
    wd = np.zeros((128, 36, 2), np.float16)
    for blk in range(36):
        ox, oy = blk // 6, blk % 6
        f = co * SO**4 + ox * SO**3 + oy * SO**2 + oz * SO + ow
        wd[:M, blk, :] = dense_w[:, f].T.astype(np.float16)

    db = np.tile(dense_b[None, :].astype(np.float32), (NBC, 1))

    in_maps = []
    for core in range(8):
        xs = x[NBC * core : NBC * (core + 1), 0]      # [64, X, Y, z, w]
        t = xs.transpose(1, 3, 4, 2, 0)               # [X, z, w, Y, b]
        t = t.reshape(S, 3, P3, S, NBC)               # [X, c, p, y, b]
        xq = np.ascontiguousarray(t * XSCALE).astype(E4)
        in_maps.append({"xp": xq, "wt": wt, "cb": cb, "wd": wd, "db": db})
    return in_maps


def _run(in_maps, trace=False):
    from concourse.bass_utils import run_bass_kernel_spmd

    if "nc" not in _cache:
        _cache["nc"] = _build_nc()
    return run_bass_kernel_spmd(_cache["nc"], in_maps, list(range(8)), trace=trace)


def kernel(x, conv_w, conv_b, dense_w, dense_b, _trace=False):
    x = np.asarray(x, np.float32)
    conv_w = np.asarray(conv_w, np.float32)
    conv_b = np.asarray(conv_b, np.float32)
    dense_w = np.asarray(dense_w, np.float32)
    dense_b = np.asarray(dense_b, np.float32)

    in_maps = _build_inputs(x, conv_w, conv_b, dense_w, dense_b)
    res = _run(in_maps, trace=_trace)
    out = np.concatenate([res.results[i]["out"] for i in range(8)], axis=0)
    if _trace:
        return out, res
    return out
